# revision 1
# baseline (speedup 1.0000x reference)
# Trainium2 Bass kernel for nn_Decoder (LSTM decoder w/ teacher forcing).
#
# Math (per step t, batch B=64, hidden H=512, embed E=256, vocab V=32000):
#   gates = x_t @ W_ih.T + b_ih + h @ W_hh.T + b_hh          [B, 4H]
#   i,f,g,o = split(gates); c = sig(f)*c + sig(i)*tanh(g); h = sig(o)*tanh(c)
#   y_t = c @ W_out.T + b_out                                 [B, V]
#
# Strategy (8 cores):
#   - Vocab-parallel: W_out sharded V -> 4000/core; recurrence replicated.
#   - Embedding gather + token shift + all layout transforms on host.
#   - Per step on device: gates via 2x column-tiled matmuls (batch M=64 ->
#     both 64-col halves of the PE array used: psum parts 0:64 = (i|f) gates,
#     parts 64:128 = (g|o) gates). x-part matmuls prefilled one step ahead.
#   - Bias add fused into PSUM eviction (DVE scalar_tensor_tensor).
#   - c_t and sig(o) stacked into one [128,512] tile; 4 PE transposes yield
#     c_t^T (for the projection) and sig(o)^T -> h^T = tanh(c^T)*sig(o)^T
#     without extra transposes.
#   - Projection: c^T collected in 8-step windows; [128row x 512K x 4000N]
#     bf16 matmuls interleaved (4 pieces/step) into the next window's PE
#     stream; PSUM evicted to SBUF staging, 2MB contiguous DMA out.
import numpy as np
import ml_dtypes
from contextlib import ExitStack

SOS = 1
V, E, H = 32000, 256, 512
T, B = 64, 64
NCORES = 8
VS = V // NCORES          # 4000 vocab shard per core
ROWS = T * B              # 4096
G = 4 * H                 # 2048
WIN = 8                   # steps per projection window
NWIN = T // WIN           # 8
NCH = 8                   # vocab chunks per row tile
NW = VS // NCH            # 500 (one PSUM bank)

_compiled_nc = None


def _build():
    import concourse.bacc as bacc
    import concourse.tile as tile
    from concourse import mybir
    from concourse.masks import make_identity

    f32 = mybir.dt.float32
    bf16 = mybir.dt.bfloat16
    AFT = mybir.ActivationFunctionType
    Alu = mybir.AluOpType

    nc = bacc.Bacc(None)
    d_xT = nc.declare_dram_parameter("xT", [128, 2, ROWS], bf16, isOutput=False)
    d_wih = nc.declare_dram_parameter("wihT", [128, 2, G], bf16, isOutput=False)
    d_whh = nc.declare_dram_parameter("whhT", [128, 4, G], f32, isOutput=False)
    d_bias = nc.declare_dram_parameter("biasg", [128, 1024], f32, isOutput=False)
    d_wout = nc.declare_dram_parameter("woutT", [128, 4, VS], bf16, isOutput=False)
    d_h0T = nc.declare_dram_parameter("h0T", [128, 4, B], f32, isOutput=False)
    d_c0 = nc.declare_dram_parameter("c0", [B, H], f32, isOutput=False)
    d_y = nc.declare_dram_parameter("yout", [ROWS, VS], f32, isOutput=True)

    with tile.TileContext(nc) as tc, ExitStack() as ctx:
        consts = ctx.enter_context(tc.tile_pool(name="consts", bufs=1))
        gsb = ctx.enter_context(tc.tile_pool(name="gsb", bufs=2))
        actp = ctx.enter_context(tc.tile_pool(name="actp", bufs=2))
        stp = ctx.enter_context(tc.tile_pool(name="stp", bufs=2))
        htp = ctx.enter_context(tc.tile_pool(name="htp", bufs=2))
        trt = ctx.enter_context(tc.tile_pool(name="trt", bufs=2))
        ctp = ctx.enter_context(tc.tile_pool(name="ctp", bufs=2))
        yp = ctx.enter_context(tc.tile_pool(name="yp", bufs=2))
        gps = ctx.enter_context(tc.tile_pool(name="gps", bufs=2, space="PSUM"))
        trpsp = ctx.enter_context(tc.tile_pool(name="trpsp", bufs=2, space="PSUM"))
        pps = ctx.enter_context(tc.tile_pool(name="pps", bufs=2, space="PSUM"))

        xT = consts.tile([128, 2, ROWS], bf16, name="xT_sb")
        nc.sync.dma_start(out=xT, in_=d_xT[:, :, :])
        wih = consts.tile([128, 2, G], bf16, name="wih_sb")
        nc.sync.dma_start(out=wih, in_=d_wih[:, :, :])
        whh = consts.tile([128, 4, G], f32, name="whh_sb")
        nc.sync.dma_start(out=whh, in_=d_whh[:, :, :])
        bias = consts.tile([128, 1024], f32, name="bias_sb")
        nc.sync.dma_start(out=bias, in_=d_bias[:, :])
        wout = consts.tile([128, 4, VS], bf16, name="wout_sb")
        nc.sync.dma_start(out=wout, in_=d_wout[:, :, :])
        ident = consts.tile([128, 128], f32, name="ident")
        make_identity(nc, ident)
        h0T = consts.tile([128, 4, B], f32, name="h0T_sb")
        nc.sync.dma_start(out=h0T, in_=d_h0T[:, :, :])
        c0 = consts.tile([B, H], f32, name="c0_sb")
        nc.sync.dma_start(out=c0, in_=d_c0[:, :])

        CT = {}
        Ysb = {}

        def emit_x(t, ps):
            # x_t part of gates (+1 step ahead of the recurrence).
            # col-tile A -> psum parts 0:64 covers gate dims [0,1024) = i|f
            # col-tile B -> psum parts 64:128 covers gate dims [1024,2048) = g|o
            for n in range(2):
                for k in range(2):
                    st = k == 0
                    lhs = xT[:, k, t * B:(t + 1) * B]
                    nc.tensor.matmul(
                        ps[0:64, n * 512:(n + 1) * 512], lhs,
                        wih[:, k, n * 512:(n + 1) * 512],
                        start=st, stop=False, skip_group_check=True)
                    nc.tensor.matmul(
                        ps[64:128, n * 512:(n + 1) * 512], lhs,
                        wih[:, k, 1024 + n * 512:1024 + (n + 1) * 512],
                        start=st, stop=False, skip_group_check=True)

        def emit_h(ps, hT):
            for n in range(2):
                for k in range(4):
                    sp = k == 3
                    lhs = hT[:, k, :]
                    nc.tensor.matmul(
                        ps[0:64, n * 512:(n + 1) * 512], lhs,
                        whh[:, k, n * 512:(n + 1) * 512],
                        start=False, stop=sp, skip_group_check=True)
                    nc.tensor.matmul(
                        ps[64:128, n * 512:(n + 1) * 512], lhs,
                        whh[:, k, 1024 + n * 512:1024 + (n + 1) * 512],
                        start=False, stop=sp, skip_group_check=True)

        def emit_proj_pieces(wp, s):
            # 4 of the 32 (rowtile, vocab-chunk) pieces of window wp
            for j in range(4):
                pi = s * 4 + j
                rt = pi // NCH
                n = pi % NCH
                if n == 0:
                    Ysb[(wp, rt)] = yp.tile([128, VS], f32, name="Ystage")
                pp = pps.tile([128, NW], f32, name="pp")
                for k in range(4):
                    nc.tensor.matmul(
                        pp, CT[wp][:, k, rt * 128:(rt + 1) * 128],
                        wout[:, k, n * NW:(n + 1) * NW],
                        start=(k == 0), stop=(k == 3))
                Y = Ysb[(wp, rt)]
                if pi % 2 == 0:
                    nc.vector.tensor_copy(out=Y[:, n * NW:(n + 1) * NW], in_=pp)
                else:
                    nc.scalar.copy(out=Y[:, n * NW:(n + 1) * NW], in_=pp)
                if n == NCH - 1:
                    r0 = wp * 512 + rt * 128
                    nc.sync.dma_start(out=d_y[r0:r0 + 128, :], in_=Y)
                    del Ysb[(wp, rt)]

        hT_prev = h0T
        c_prev = c0
        psg = {0: gps.tile([128, 1024], f32, name="psg")}
        emit_x(0, psg[0])

        for t in range(T):
            w = t // WIN
            s = t % WIN
            emit_h(psg[t], hT_prev)
            if t + 1 < T:
                psg[t + 1] = gps.tile([128, 1024], f32, name="psg")
                emit_x(t + 1, psg[t + 1])
            if w >= 1:
                emit_proj_pieces(w - 1, s)

            ps = psg.pop(t)
            gates = gsb.tile([128, 1024], f32, name="gates")
            nc.vector.scalar_tensor_tensor(
                out=gates[:, 0:512], in0=ps[:, 0:512], scalar=1.0,
                in1=bias[:, 0:512], op0=Alu.mult, op1=Alu.add)
            nc.vector.scalar_tensor_tensor(
                out=gates[:, 512:1024], in0=ps[:, 512:1024], scalar=1.0,
                in1=bias[:, 512:1024], op0=Alu.mult, op1=Alu.add)
            # parts 0:64 of gates = (i|f), parts 64:128 = (g|o)
            sif = actp.tile([64, 1024], f32, name="sif")
            nc.scalar.activation(out=sif, in_=gates[0:64, :], func=AFT.Sigmoid)
            tg = actp.tile([64, 512], f32, name="tg")
            nc.scalar.activation(out=tg, in_=gates[64:128, 0:512], func=AFT.Tanh)
            t1 = actp.tile([64, 512], f32, name="t1")
            nc.vector.tensor_mul(t1, sif[:, 0:512], tg)
            t2 = actp.tile([64, 512], f32, name="t2")
            nc.vector.tensor_mul(t2, sif[:, 512:1024], c_prev)
            stacked = stp.tile([128, 512], f32, name="stacked")
            nc.vector.tensor_add(stacked[0:64, :], t1, t2)  # c_t
            nc.scalar.activation(
                out=stacked[64:128, :], in_=gates[64:128, 512:1024],
                func=AFT.Sigmoid)  # sig(o)

            trp = trpsp.tile([128, 4, 128], f32, name="trp")
            for j in range(4):
                nc.tensor.matmul(
                    trp[:, j, :], stacked[:, j * 128:(j + 1) * 128], ident,
                    is_transpose=True, start=True, stop=True,
                    skip_group_check=True)

            if s == 0:
                CT[w] = ctp.tile([128, 4, WIN * B], bf16, name="CT")
            nc.vector.tensor_copy(
                out=CT[w][:, :, s * 64:(s + 1) * 64], in_=trp[:, :, 0:64])
            if t + 1 < T:
                tcT = trt.tile([128, 4, 64], f32, name="tcT")
                nc.scalar.activation(out=tcT, in_=trp[:, :, 0:64], func=AFT.Tanh)
                soT = trt.tile([128, 4, 64], f32, name="soT")
                nc.vector.tensor_copy(out=soT, in_=trp[:, :, 64:128])
                hT_new = htp.tile([128, 4, 64], f32, name="hT")
                nc.vector.tensor_mul(hT_new, tcT, soT)
                hT_prev = hT_new
            c_prev = stacked[0:64, :]

        for s in range(WIN):
            emit_proj_pieces(NWIN - 1, s)

    nc.finalize()
    return nc


def _get_compiled():
    global _compiled_nc
    if _compiled_nc is None:
        _compiled_nc = _build()
    return _compiled_nc


def kernel(**inputs):
    from concourse.bass_utils import run_bass_kernel_spmd

    h = np.asarray(inputs["h"], dtype=np.float32)
    c = np.asarray(inputs["c"], dtype=np.float32)
    y = np.asarray(inputs["y"])
    emb = np.asarray(inputs["embed_table"], dtype=np.float32)
    W_ih = np.asarray(inputs["W_ih"], dtype=np.float32)
    b_ih = np.asarray(inputs["b_ih"], dtype=np.float32)
    W_hh = np.asarray(inputs["W_hh"], dtype=np.float32)
    b_hh = np.asarray(inputs["b_hh"], dtype=np.float32)
    W_out = np.asarray(inputs["W_out"], dtype=np.float32)
    b_out = np.asarray(inputs["b_out"], dtype=np.float32)

    tokens = np.concatenate(
        [np.full((1, B), SOS, dtype=y.dtype), y[:-1]], axis=0)
    X2 = emb[tokens].reshape(ROWS, E)  # row r = t*B + b

    xT_np = np.ascontiguousarray(
        X2.T.reshape(2, 128, ROWS).transpose(1, 0, 2)).astype(ml_dtypes.bfloat16)
    wih_np = np.ascontiguousarray(
        W_ih.T.reshape(2, 128, G).transpose(1, 0, 2)).astype(ml_dtypes.bfloat16)
    whh_np = np.ascontiguousarray(
        W_hh.T.reshape(4, 128, G).transpose(1, 0, 2)).astype(np.float32)
    bvec = b_ih + b_hh
    bias_np = np.empty((128, 1024), dtype=np.float32)
    bias_np[0:64, :] = bvec[None, 0:1024]
    bias_np[64:128, :] = bvec[None, 1024:2048]
    h0T_np = np.ascontiguousarray(
        h.T.reshape(4, 128, B).transpose(1, 0, 2)).astype(np.float32)

    in_maps = []
    for ci in range(NCORES):
        Wsh = W_out[ci * VS:(ci + 1) * VS, :]
        wout_np = np.ascontiguousarray(
            Wsh.T.reshape(4, 128, VS).transpose(1, 0, 2)).astype(ml_dtypes.bfloat16)
        in_maps.append({
            "xT": xT_np, "wihT": wih_np, "whhT": whh_np, "biasg": bias_np,
            "woutT": wout_np, "h0T": h0T_np, "c0": c,
        })

    nc = _get_compiled()
    res = run_bass_kernel_spmd(nc, in_maps, list(range(NCORES))).results
    Y = np.concatenate(
        [np.asarray(r["yout"]).reshape(T, B, VS) for r in res], axis=2)
    if np.any(b_out != 0):
        Y = Y + b_out[None, None, :]
    return np.ascontiguousarray(Y.astype(np.float32))
